# revision 6
# baseline (speedup 1.0000x reference)
"""Trainium2 Bass kernel for multi-head attention (dense_transformer).

Full module: qkv = x @ W_qkv + b_qkv; multi-head attention (16 heads, d=64,
N=4096); out = attn @ W_proj + b_proj.

Sharding: tensor-parallel over heads — 2 heads per core on 8 cores. Each core
receives full x (pre-transposed on host to [C, N]) plus its head-slices of the
weights, computes its heads' attention and a partial output projection
[N, C]; the host sums the 8 partials and adds b_proj.

Per-core dataflow (all matmuls in float32r — full PE rate, ~tf32 precision):
  A) Q^T,K^T [128, 4096] = W^T @ x^T accumulated over C chunks (PSUM), bias
     added on eviction.  V^T likewise, then PE-transposed to V natural
     [tok, d] stored with a constant ones column per head: [V_h | 1].
  B) per (q-chunk 512, k-chunk 128): S^T [128, 1024] for both heads packed
     side by side (row-tiled K=64 matmuls), ACT exp (scale=1/8) from PSUM to
     SBUF f32r, then AV matmuls lhsT=[V_h|1] accumulate out^T[65, 512] whose
     row 64 is the softmax denominator.  Normalize with DVE reciprocal +
     GPSIMD partition broadcast + DVE mul, add b_v.
  C) partial proj: out[tok,  C] = attn_out^T.T @ W_proj_slice, evicted by DVE
     and DMAd out.
"""

import numpy as np
from contextlib import ExitStack

NUM_CORES = 8
DIM = 1024
NUM_HEADS = 16
HDIM = 64
N = 4096
HPC = NUM_HEADS // NUM_CORES   # heads per core = 2
DPC = HPC * HDIM               # head dims per core = 128

_NC_CACHE = {}


def build_nc(reps=1):
    if reps in _NC_CACHE:
        return _NC_CACHE[reps]

    import concourse.bass as bass
    import concourse.mybir as mybir
    import concourse.tile as tile
    from concourse import bacc
    from concourse.masks import make_identity

    f32 = mybir.dt.float32
    f32r = mybir.dt.float32r
    AF = mybir.ActivationFunctionType
    ts = bass.ts

    nc = bacc.Bacc(trn_type="TRN2", target_bir_lowering=False, debug=False)
    xT = nc.dram_tensor("xT", [DIM, N], f32r, kind="ExternalInput").ap()
    wq = nc.dram_tensor("wq", [DIM, DPC], f32r, kind="ExternalInput").ap()
    wk = nc.dram_tensor("wk", [DIM, DPC], f32r, kind="ExternalInput").ap()
    wv = nc.dram_tensor("wv", [DIM, DPC], f32r, kind="ExternalInput").ap()
    wp = nc.dram_tensor("wp", [DPC, DIM], f32r, kind="ExternalInput").ap()
    bq = nc.dram_tensor("bq", [DPC, 1], f32, kind="ExternalInput").ap()
    bk = nc.dram_tensor("bk", [DPC, 1], f32, kind="ExternalInput").ap()
    bv = nc.dram_tensor("bv", [DPC, 1], f32, kind="ExternalInput").ap()
    ones = nc.dram_tensor("ones", [1, 1], f32r, kind="ExternalInput").ap()
    out = nc.dram_tensor("out", [N, DIM], f32, kind="ExternalOutput").ap()

    with tile.TileContext(nc) as tc, ExitStack() as ctx:
        singles = ctx.enter_context(tc.tile_pool(name="singles", bufs=1))
        psum = ctx.enter_context(tc.tile_pool(name="ps", bufs=2, space="PSUM"))
        xpool = ctx.enter_context(tc.tile_pool(name="xp", bufs=2))
        work = ctx.enter_context(tc.tile_pool(name="work", bufs=2))
        ppool = ctx.enter_context(tc.tile_pool(name="pp", bufs=3))
        opool = ctx.enter_context(tc.tile_pool(name="op", bufs=3))

        ident = singles.tile([128, 128], f32, tag="ident")
        make_identity(nc, ident)

        wq_sb = singles.tile([128, 8, DPC], f32r, tag="wq")
        wk_sb = singles.tile([128, 8, DPC], f32r, tag="wk")
        wv_sb = singles.tile([128, 8, DPC], f32r, tag="wv")
        nc.sync.dma_start(out=wq_sb, in_=wq.rearrange("(c p) m -> p c m", p=128))
        nc.sync.dma_start(out=wk_sb, in_=wk.rearrange("(c p) m -> p c m", p=128))
        nc.sync.dma_start(out=wv_sb, in_=wv.rearrange("(c p) m -> p c m", p=128))
        wp_sb = singles.tile([64, HPC, DIM], f32r, tag="wp")
        nc.sync.dma_start(out=wp_sb, in_=wp.rearrange("(h d) c -> d h c", d=64))
        bq_sb = singles.tile([DPC, 1], f32, tag="bq")
        bk_sb = singles.tile([DPC, 1], f32, tag="bk")
        nc.sync.dma_start(out=bq_sb, in_=bq)
        nc.sync.dma_start(out=bk_sb, in_=bk)
        bv_sb = singles.tile([64, HPC, 1], f32, tag="bv")
        nc.sync.dma_start(out=bv_sb, in_=bv.rearrange("(h d) x -> d h x", d=64))

        qT = singles.tile([128, N], f32r, tag="qT")
        kT = singles.tile([128, N], f32r, tag="kT")
        aoT0 = singles.tile([64, N], f32r, tag="aoT0")
        aoT1 = singles.tile([64, N], f32r, tag="aoT1")
        # V natural layout + ones column per head: [.., t, 0:64]=V_h0,
        # [.., t, 64]=1, [.., t, 65:129]=V_h1, [.., t, 129]=1
        v_nat = singles.tile([128, 32, 130], f32r, tag="vnat")
        # ones columns loaded via broadcast DMA (memset can't target f32r)
        nc.sync.dma_start(out=v_nat[:, :, 64:65], in_=ones.to_broadcast((128, 32, 1)))
        nc.sync.dma_start(out=v_nat[:, :, 129:130], in_=ones.to_broadcast((128, 32, 1)))

        for _rep in range(reps):
            # ---------------- Phase A: QKV projection ----------------
            for qt in range(4):
                xt = [xpool.tile([128, 1024], f32r, tag=f"x{c}", name=f"x{c}") for c in range(8)]
                for c in range(8):
                    nc.sync.dma_start(out=xt[c], in_=xT[ts(c, 128), ts(qt, 1024)])
                for nl in range(2):
                    n = qt * 2 + nl
                    for wsb, dest, bias in ((wq_sb, qT, bq_sb), (wk_sb, kT, bk_sb)):
                        acc = psum.tile([128, 512], f32, tag="big")
                        for c in range(8):
                            nc.tensor.matmul(
                                acc, wsb[:, c, :], xt[c][:, ts(nl, 512)],
                                start=(c == 0), stop=(c == 7),
                            )
                        nc.vector.tensor_scalar_add(dest[:, ts(n, 512)], acc, bias)
                    vacc = psum.tile([128, 512], f32, tag="big")
                    for c in range(8):
                        nc.tensor.matmul(
                            vacc, wv_sb[:, c, :], xt[c][:, ts(nl, 512)],
                            start=(c == 0), stop=(c == 7),
                        )
                    vst = work.tile([128, 512], f32, tag="vst")
                    nc.vector.tensor_copy(vst, vacc)
                    for tl in range(4):
                        t = n * 4 + tl
                        tp = psum.tile(
                            [128, 128], f32, tag=("av0" if tl % 2 == 0 else "av1"), name="tp"
                        )
                        nc.tensor.transpose(tp, vst[:, ts(tl, 128)], ident)
                        nc.vector.tensor_copy(
                            out=v_nat[:, t, 0:130]
                            .rearrange("p (g d) -> p g d", d=65)[:, :, 0:64],
                            in_=tp.rearrange("p (g d) -> p g d", d=64),
                        )

            # ---------------- Phase B: attention ----------------
            for qi in range(8):
                av = [
                    psum.tile([65, 512], f32, tag="av0", name="av0"),
                    psum.tile([65, 512], f32, tag="av1", name="av1"),
                ]
                # software-pipelined: emit scores/exp one step ahead of AV
                s_tiles = {}
                p_tiles = {}
                for ki in range(33):
                    if ki < 32:
                        s = psum.tile([128, 1024], f32, tag="big")
                        nc.tensor.matmul(
                            s[:, 0:512], kT[0:64, ts(ki, 128)], qT[0:64, ts(qi, 512)],
                            start=True, stop=True,
                        )
                        nc.tensor.matmul(
                            s[:, 512:1024], kT[64:128, ts(ki, 128)],
                            qT[64:128, ts(qi, 512)],
                            start=True, stop=True,
                        )
                        p = ppool.tile([128, 1024], f32r, tag="p")
                        nc.scalar.activation(p, s, AF.Exp, scale=0.125)
                        p_tiles[ki] = p
                    if ki >= 1:
                        kj = ki - 1
                        p = p_tiles.pop(kj)
                        nc.tensor.matmul(
                            av[0], v_nat[:, kj, 0:65], p[:, 0:512],
                            start=(kj == 0), stop=(kj == 31),
                        )
                        nc.tensor.matmul(
                            av[1], v_nat[:, kj, 65:130], p[:, 512:1024],
                            start=(kj == 0), stop=(kj == 31),
                        )
                for h, (acc, aoT) in enumerate(((av[0], aoT0), (av[1], aoT1))):
                    recip = work.tile([1, 512], f32, tag="recip")
                    nc.vector.reciprocal(recip, acc[64:65, :])
                    bc = work.tile([64, 512], f32, tag="bc")
                    nc.gpsimd.partition_broadcast(bc, recip)
                    nc.vector.tensor_mul(aoT[:, ts(qi, 512)], acc[0:64, :], bc)
                    nc.vector.tensor_scalar_add(
                        aoT[:, ts(qi, 512)], aoT[:, ts(qi, 512)], bv_sb[:, h, :]
                    )

            # ---------------- Phase C: output projection (partial) -------
            for t in range(32):
                for j in range(2):
                    pp = psum.tile([128, 512], f32, tag=("av0" if j == 0 else "av1"), name="pp")
                    nc.tensor.matmul(
                        pp, aoT0[:, ts(t, 128)], wp_sb[:, 0, ts(j, 512)],
                        start=True, stop=False,
                    )
                    nc.tensor.matmul(
                        pp, aoT1[:, ts(t, 128)], wp_sb[:, 1, ts(j, 512)],
                        start=False, stop=True,
                    )
                    ot = opool.tile([128, 512], f32, tag="ot")
                    nc.vector.tensor_copy(ot, pp)
                    nc.sync.dma_start(out=out[ts(t, 128), ts(j, 512)], in_=ot)

    nc.compile()
    _NC_CACHE[reps] = nc
    return nc


def make_in_maps(x, W_qkv, b_qkv, W_proj):
    x2 = np.asarray(x, dtype=np.float32).reshape(N, DIM)
    xTv = np.ascontiguousarray(x2.T)
    W_qkv = np.asarray(W_qkv, dtype=np.float32)
    b_qkv = np.asarray(b_qkv, dtype=np.float32)
    W_proj = np.asarray(W_proj, dtype=np.float32)
    maps = []
    for m in range(NUM_CORES):
        h0 = m * DPC
        maps.append({
            "xT": xTv,
            "wq": np.ascontiguousarray(W_qkv[:, h0:h0 + DPC]),
            "wk": np.ascontiguousarray(W_qkv[:, DIM + h0:DIM + h0 + DPC]),
            "wv": np.ascontiguousarray(W_qkv[:, 2 * DIM + h0:2 * DIM + h0 + DPC]),
            "wp": np.ascontiguousarray(W_proj[h0:h0 + DPC, :]),
            "bq": np.ascontiguousarray(b_qkv[h0:h0 + DPC].reshape(DPC, 1)),
            "bk": np.ascontiguousarray(
                b_qkv[DIM + h0:DIM + h0 + DPC].reshape(DPC, 1)),
            "bv": np.ascontiguousarray(
                b_qkv[2 * DIM + h0:2 * DIM + h0 + DPC].reshape(DPC, 1)),
            "ones": np.ones((1, 1), dtype=np.float32),
        })
    return maps


def kernel(x, W_qkv, b_qkv, W_proj, b_proj, _reps=1):
    from concourse.bass_utils import run_bass_kernel_spmd

    nc = build_nc(_reps)
    maps = make_in_maps(x, W_qkv, b_qkv, W_proj)
    res = run_bass_kernel_spmd(nc, maps, list(range(NUM_CORES)))
    partial = np.stack([r["out"] for r in res.results], axis=0)
    total = partial.sum(axis=0, dtype=np.float32)
    total = total + np.asarray(b_proj, dtype=np.float32)[None, :]
    return total.reshape(1, N, DIM).astype(np.float32)
